# revision 8
# baseline (speedup 1.0000x reference)
"""Bit-packed binary (masked-XNOR popcount) matmul on 8 TRN2 NeuronCores.

Math: acc[p,b,o] = C[p,o] + sum_k x_bit[b,k] * W[p,k,o], W = m*(2s-1) in
{-1,0,+1}, C[p,o] = sum_k m*(1-s).  P=16 is sharded 2-per-core.

v2 strategy ("packed codes"): instead of shipping W as fp8 (1 byte per
weight, 32MB/core, DMA-bound at ~94us), the host packs FOUR ternary
weights per byte as 2-bit fields using codes {00->0, 01->+1, 11->-1}
placed at bits {7,6},{5,4},{3,2},{1,0}.  On-chip, plane f is recovered
with a single fused tensor_scalar op
    plane_f = (packed << 2f) & 0xC0C0C0C0
whose output bytes are *directly* valid fp8e4m3 values {0x00->0,
0x40->+2, 0xC0->-2} = 2*w.  The stationary x is pre-scaled by 0.5 so
PSUM accumulates exact integers sum x*w.  W traffic drops to 8MB/core
(~24us), below the PE floor, and the DVE/Pool engines absorb the
expansion (~32M fp8 bytes) in parallel with the PE.

Layouts (per core):
  x host  [kk=128, kcp=16, j=2, b=128]  fp8 = 0.5*bit   (k = kcp*256+j*128+kk)
  wp host [grp=4, kk=128, q=8, col=2048] int8 packed codes,
          grp = pl*2 + h (o = h*2048 + col);
          byte field f in {0..3} <- weight (kcp = 2q + (f>>1), j = f&1)
  out     [pl, h, b, col] int8 (matmul part only; C re-centers on host)
"""

import numpy as np
import ml_dtypes

B = 128
I = 64
K = 4096
O = 4096
P = 16
NCORES = 8
PL = P // NCORES   # 2
OH = 2
OHW = O // OH      # 2048
KCP = 16
NQ = 8             # packed q-slots per group
NGRP = PL * OH     # 4

# --- tunables -----------------------------------------------------------
N_MM = 512         # moving-output cols per matmul instruction
CQ = 4             # q-slots per W DMA chunk (chunk = CQ*256KB packed)
POOL_TT = False    # bitwise int32 ops are DVE-only (walrus NCC_EBIR039)
# ------------------------------------------------------------------------

F8 = ml_dtypes.float8_e4m3

_CACHE = {}


def _build_nc():
    import concourse.bass as bass
    import concourse.mybir as mybir
    import concourse.tile as tile
    from concourse import bacc

    fp8 = mybir.dt.float8e4
    f32 = mybir.dt.float32
    i8 = mybir.dt.int8
    i32 = mybir.dt.int32
    DR = mybir.MatmulPerfMode.DoubleRow
    MASK = np.int32(np.uint32(0xC0C0C0C0)).item()
    SL = mybir.AluOpType.logical_shift_left
    AND = mybir.AluOpType.bitwise_and

    NCH = NQ // CQ            # chunks per group
    NC_K = 2 * CQ             # kcp slots per chunk
    NSUB = OHW // N_MM        # matmuls per kcp

    XA = 4                    # kcp slots in the small leading x tile
    # group-0 chunk schedule (q0, nq): small leading chunks so the first
    # wt tile (its own tile -> its own DMA/extract deps) lands early.
    G0_CHUNKS = [(0, 1), (1, 1), (2, 2), (4, 4)]
    GN_CHUNKS = [(0, 4), (4, 4)]

    nc = bacc.Bacc("TRN2", target_bir_lowering=False)
    xs_d = nc.dram_tensor("xs", [128, KCP, 2, B], fp8, kind="ExternalInput")
    wp_d = nc.dram_tensor("wp", [NGRP, 128, NQ, OHW], i8, kind="ExternalInput")
    out_d = nc.dram_tensor("out", [NGRP, B, OHW], i8, kind="ExternalOutput")

    with tile.TileContext(nc) as tc:
        with (
            tc.tile_pool(name="xpa", bufs=1) as xpa,
            tc.tile_pool(name="xpb", bufs=1) as xpb,
            tc.tile_pool(name="pk1", bufs=2) as pk1,
            tc.tile_pool(name="wt1", bufs=2) as wt1,
            tc.tile_pool(name="pk2", bufs=1) as pk2,
            tc.tile_pool(name="wt2", bufs=1) as wt2,
            tc.tile_pool(name="pk4", bufs=3) as pk4,
            tc.tile_pool(name="wt4", bufs=2) as wt4,
            tc.tile_pool(name="pp", bufs=2, space=bass.MemorySpace.PSUM) as pp,
            tc.tile_pool(name="op", bufs=2) as op_,
        ):
            xsa = xpa.tile([128, XA, 2, B], fp8)
            xsb = xpb.tile([128, KCP - XA, 2, B], fp8)
            nc.sync.dma_start(xsa[:], xs_d[:, 0:XA])
            dma_rr = [nc.scalar, nc.sync]
            n_dma = 0
            pools = {1: (pk1, wt1), 2: (pk2, wt2), 4: (pk4, wt4)}

            def xap(kcp):
                return xsa[:, kcp] if kcp < XA else xsb[:, kcp - XA]

            def extract(wp32, wt32, nq):
                for f in range(4):
                    lo = f >> 1
                    hi = lo + 2 * nq - 1
                    dst = wt32[:, lo:hi if hi > lo else lo + 1:2, (f & 1), :]
                    if f == 0:
                        nc.vector.tensor_scalar(dst, wp32, MASK, None, AND)
                    else:
                        nc.vector.tensor_scalar(dst, wp32, 2 * f, MASK, SL,
                                                AND)

            first_emitted = [False]

            for g in range(NGRP):
                ps = pp.tile([128, OHW], f32)
                chunks = G0_CHUNKS if g == 0 else GN_CHUNKS
                for (q0, nq) in chunks:
                    pk, wtp = pools[nq]
                    wpk = pk.tile([128, nq, OHW], i8)
                    wt = wtp.tile([128, 2 * nq, 2, OHW], fp8)
                    wt32 = wt[:].bitcast(i32)
                    wp32 = wpk[:].bitcast(i32)
                    eng = dma_rr[n_dma % 2]
                    n_dma += 1
                    eng.dma_start(wpk[:], wp_d[g, :, q0:q0 + nq])
                    if not first_emitted[0]:
                        # xsb after the first small W chunk on the other ring
                        nc.sync.dma_start(xsb[:], xs_d[:, XA:])
                        first_emitted[0] = True
                    extract(wp32, wt32, nq)
                    last_chunk = (g == NGRP - 1) and (q0 + nq == NQ)
                    if last_chunk:
                        ot = op_.tile([128, OHW], i8)
                    for kl in range(2 * nq):
                        kcp = 2 * q0 + kl
                        for oc in range(NSUB):
                            sl = slice(oc * N_MM, (oc + 1) * N_MM)
                            nc.tensor.matmul(
                                ps[:, sl], xap(kcp), wt[:, kl, :, sl],
                                start=(kcp == 0), stop=(kcp == KCP - 1),
                                perf_mode=DR)
                            if last_chunk and kcp == KCP - 1:
                                # per-slice eviction chained to stop-matmul
                                nc.scalar.copy(ot[:, sl], ps[:, sl])
                                nc.sync.dma_start(out_d[g, :, sl], ot[:, sl])
                if g < NGRP - 1:
                    ot = op_.tile([128, OHW], i8)
                    nc.scalar.copy(ot[:], ps[:])
                    nc.gpsimd.dma_start(out_d[g], ot[:])

    nc.compile()
    return nc


def _unpack_inputs(x, w):
    """Host-side: x bits -> fp8 stationary (0.5*bit); W -> packed 2-bit
    codes; popcount bias C."""
    xbits = np.unpackbits(
        np.ascontiguousarray(x).view(np.uint8).reshape(B, I * 8),
        axis=1, bitorder="little",
    )  # [B, K] in {0,1}
    xr = xbits.T.reshape(KCP, 2, 128, B)              # [kcp, j, kk, b]
    xs = np.ascontiguousarray(
        (0.5 * xr.transpose(2, 0, 1, 3)).astype(np.float32)).astype(F8)

    s_words = np.ascontiguousarray(w[0])  # [P, I, O] int64
    m_words = np.ascontiguousarray(w[1])

    wp_all = np.empty((P, OH, 128, NQ, OHW), np.uint8)
    C = np.empty((P, O), np.int32)
    for p in range(P):
        sb = np.unpackbits(
            s_words[p].view(np.uint8).reshape(I, O, 8), axis=2,
            bitorder="little").transpose(0, 2, 1).reshape(K, O)
        mb = np.unpackbits(
            m_words[p].view(np.uint8).reshape(I, O, 8), axis=2,
            bitorder="little").transpose(0, 2, 1).reshape(K, O)
        C[p] = (mb * (1 - sb)).astype(np.int32).sum(axis=0)
        # codes: 0 -> 0b00, +1 -> 0b01, -1 -> 0b11  (w = m*(2s-1))
        code = (mb * (1 + 2 * (1 - sb))).astype(np.uint8)  # +1->1, -1->3
        # [K, O] -> [kcp, j, kk, h, col] -> fields
        cr = code.reshape(KCP, 2, 128, OH, OHW)
        c4 = cr.reshape(NQ, 2, 2, 128, OH, OHW)  # [q, kcp_sub, j, kk, h, col]
        byte = ((c4[:, 0, 0] << 6) | (c4[:, 0, 1] << 4)
                | (c4[:, 1, 0] << 2) | c4[:, 1, 1])   # [q, kk, h, col]
        wp_all[p] = byte.transpose(2, 1, 0, 3)        # [h, kk, q, col]
    return xs, wp_all, C


def _run(nc, in_maps, trace=False):
    from concourse import bass_utils
    return bass_utils.run_bass_kernel_spmd(
        nc, in_maps, core_ids=list(range(NCORES)), trace=trace
    )


def kernel(x, w, _trace=False, _return_results=False):
    x = np.asarray(x)
    w = np.asarray(w)
    assert x.shape == (B, I) and w.shape == (2, P, I, O)

    xs, wp_all, C = _unpack_inputs(x, w)

    if "nc" not in _CACHE:
        _CACHE["nc"] = _build_nc()
    nc = _CACHE["nc"]

    in_maps = []
    for c in range(NCORES):
        # groups for core c: [pl, h] -> wp_all[2c+pl, h]
        wp = np.ascontiguousarray(
            wp_all[2 * c:2 * c + PL].reshape(NGRP, 128, NQ, OHW)
        ).view(np.int8)
        in_maps.append({"xs": xs, "wp": wp})
    res = _run(nc, in_maps, trace=_trace)

    out = np.empty((P, B, O), np.int32)
    for c in range(NCORES):
        o = res.results[c]["out"]  # [NGRP, B, OHW] int8
        for pl in range(PL):
            full = np.concatenate(
                [o[pl * OH], o[pl * OH + 1]], axis=1)  # [B, O]
            out[c * PL + pl] = full.astype(np.int32) + C[c * PL + pl][None, :]
    if _return_results:
        return out, res
    return out


# revision 12
# speedup vs baseline: 1.0377x; 1.0377x over previous
"""Bit-packed binary (masked-XNOR popcount) matmul on 8 TRN2 NeuronCores.

Math: acc[p,b,o] = C[p,o] + sum_k x_bit[b,k] * W[p,k,o], W = m*(2s-1) in
{-1,0,+1}, C[p,o] = sum_k m*(1-s).  P=16 is sharded 2-per-core.

v2 strategy ("packed codes"): instead of shipping W as fp8 (1 byte per
weight, 32MB/core, DMA-bound at ~94us), the host packs FOUR ternary
weights per byte as 2-bit fields using codes {00->0, 01->+1, 11->-1}
placed at bits {7,6},{5,4},{3,2},{1,0}.  On-chip, plane f is recovered
with a single fused tensor_scalar op
    plane_f = (packed << 2f) & 0xC0C0C0C0
whose output bytes are *directly* valid fp8e4m3 values {0x00->0,
0x40->+2, 0xC0->-2} = 2*w.  The stationary x is pre-scaled by 0.5 so
PSUM accumulates exact integers sum x*w.  W traffic drops to 8MB/core
(~24us), below the PE floor, and the DVE/Pool engines absorb the
expansion (~32M fp8 bytes) in parallel with the PE.

Layouts (per core):
  x host  [kk=128, kcp=16, j=2, b=128]  fp8 = 0.5*bit   (k = kcp*256+j*128+kk)
  wp host [grp=4, kk=128, q=8, col=2048] int8 packed codes,
          grp = pl*2 + h (o = h*2048 + col);
          byte field f in {0..3} <- weight (kcp = 2q + (f>>1), j = f&1)
  out     [pl, h, b, col] int8 (matmul part only; C re-centers on host)
"""

import numpy as np
import ml_dtypes

B = 128
I = 64
K = 4096
O = 4096
P = 16
NCORES = 8
PL = P // NCORES   # 2
OH = 2
OHW = O // OH      # 2048
KCP = 16
NQ = 8             # packed q-slots per group
NGRP = PL * OH     # 4

# --- tunables -----------------------------------------------------------
N_MM = 512         # moving-output cols per matmul instruction
CQ = 4             # q-slots per W DMA chunk (chunk = CQ*256KB packed)
POOL_TT = False    # bitwise int32 ops are DVE-only (walrus NCC_EBIR039)
# ------------------------------------------------------------------------

F8 = ml_dtypes.float8_e4m3

_CACHE = {}


def _build_nc():
    import concourse.bass as bass
    import concourse.mybir as mybir
    import concourse.tile as tile
    from concourse import bacc

    fp8 = mybir.dt.float8e4
    f32 = mybir.dt.float32
    i8 = mybir.dt.int8
    i32 = mybir.dt.int32
    DR = mybir.MatmulPerfMode.DoubleRow
    MASK = np.int32(np.uint32(0xC0C0C0C0)).item()
    SL = mybir.AluOpType.logical_shift_left
    AND = mybir.AluOpType.bitwise_and

    NCH = NQ // CQ            # chunks per group
    NC_K = 2 * CQ             # kcp slots per chunk
    NSUB = OHW // N_MM        # matmuls per kcp

    XA = 4                    # kcp slots in the small leading x tile
    # group-0 chunk schedule (q0, nq, ring): small leading chunks (own
    # tiles -> own DMA/extract deps) ramping up, pinned to rings so the
    # first packed bytes land with minimal queue contention.
    G0_CHUNKS = [(0, 1, 0), (1, 1, 1), (2, 2, 0), (4, 2, 1), (6, 2, 0)]
    GN_CHUNKS = [(0, 4, None), (4, 4, None)]

    nc = bacc.Bacc("TRN2", target_bir_lowering=False)
    xs_d = nc.dram_tensor("xs", [128, KCP, 2, B], fp8, kind="ExternalInput")
    wp_d = nc.dram_tensor("wp", [NGRP, 128, NQ, OHW], i8, kind="ExternalInput")
    out_d = nc.dram_tensor("out", [NGRP, B, OHW], i8, kind="ExternalOutput")

    with tile.TileContext(nc) as tc:
        with (
            tc.tile_pool(name="xpa", bufs=1) as xpa,
            tc.tile_pool(name="xpb", bufs=1) as xpb,
            tc.tile_pool(name="pk1", bufs=2) as pk1,
            tc.tile_pool(name="wt1", bufs=2) as wt1,
            tc.tile_pool(name="pk2", bufs=1) as pk2,
            tc.tile_pool(name="wt2", bufs=1) as wt2,
            tc.tile_pool(name="pk4", bufs=3) as pk4,
            tc.tile_pool(name="wt4", bufs=2) as wt4,
            tc.tile_pool(name="pp", bufs=2, space=bass.MemorySpace.PSUM) as pp,
            tc.tile_pool(name="op", bufs=2) as op_,
        ):
            xsa = xpa.tile([128, XA, 2, B], fp8)
            xsb = xpb.tile([128, KCP - XA, 2, B], fp8)
            nc.sync.dma_start(xsa[:], xs_d[:, 0:XA])
            dma_rr = [nc.scalar, nc.sync]
            n_dma = 0
            pools = {1: (pk1, wt1), 2: (pk2, wt2), 4: (pk4, wt4)}

            def xap(kcp):
                return xsa[:, kcp] if kcp < XA else xsb[:, kcp - XA]

            def extract(wp32, wt32, nq):
                for f in range(4):
                    lo = f >> 1
                    hi = lo + 2 * nq - 1
                    dst = wt32[:, lo:hi if hi > lo else lo + 1:2, (f & 1), :]
                    if f == 0:
                        nc.vector.tensor_scalar(dst, wp32, MASK, None, AND)
                    else:
                        nc.vector.tensor_scalar(dst, wp32, 2 * f, MASK, SL,
                                                AND)

            first_emitted = [0]

            for g in range(NGRP):
                ps = pp.tile([128, OHW], f32)
                chunks = G0_CHUNKS if g == 0 else GN_CHUNKS
                for (q0, nq, ring) in chunks:
                    pk, wtp = pools[nq]
                    wpk = pk.tile([128, nq, OHW], i8)
                    wt = wtp.tile([128, 2 * nq, 2, OHW], fp8)
                    wt32 = wt[:].bitcast(i32)
                    wp32 = wpk[:].bitcast(i32)
                    if ring is None:
                        ring = n_dma % 2
                        n_dma += 1
                    dma_rr[ring].dma_start(wpk[:], wp_d[g, :, q0:q0 + nq])
                    first_emitted[0] += 1
                    if first_emitted[0] == 2:
                        # xsb behind xsa + the first sync-ring W piece
                        nc.sync.dma_start(xsb[:], xs_d[:, XA:])
                    extract(wp32, wt32, nq)
                    for kl in range(2 * nq):
                        kcp = 2 * q0 + kl
                        for oc in range(NSUB):
                            sl = slice(oc * N_MM, (oc + 1) * N_MM)
                            nc.tensor.matmul(
                                ps[:, sl], xap(kcp), wt[:, kl, :, sl],
                                start=(kcp == 0), stop=(kcp == KCP - 1),
                                perf_mode=DR)
                ot = op_.tile([128, OHW], i8)
                nc.scalar.copy(ot[:], ps[:])
                if g < NGRP - 1:
                    nc.gpsimd.dma_start(out_d[g], ot[:])
                else:
                    nc.sync.dma_start(out_d[g], ot[:])

    nc.compile()
    return nc


def _unpack_inputs(x, w):
    """Host-side: x bits -> fp8 stationary (0.5*bit); W -> packed 2-bit
    codes; popcount bias C."""
    xbits = np.unpackbits(
        np.ascontiguousarray(x).view(np.uint8).reshape(B, I * 8),
        axis=1, bitorder="little",
    )  # [B, K] in {0,1}
    xr = xbits.T.reshape(KCP, 2, 128, B)              # [kcp, j, kk, b]
    xs = np.ascontiguousarray(
        (0.5 * xr.transpose(2, 0, 1, 3)).astype(np.float32)).astype(F8)

    s_words = np.ascontiguousarray(w[0])  # [P, I, O] int64
    m_words = np.ascontiguousarray(w[1])

    wp_all = np.empty((P, OH, 128, NQ, OHW), np.uint8)
    C = np.empty((P, O), np.int32)
    for p in range(P):
        sb = np.unpackbits(
            s_words[p].view(np.uint8).reshape(I, O, 8), axis=2,
            bitorder="little").transpose(0, 2, 1).reshape(K, O)
        mb = np.unpackbits(
            m_words[p].view(np.uint8).reshape(I, O, 8), axis=2,
            bitorder="little").transpose(0, 2, 1).reshape(K, O)
        C[p] = (mb * (1 - sb)).astype(np.int32).sum(axis=0)
        # codes: 0 -> 0b00, +1 -> 0b01, -1 -> 0b11  (w = m*(2s-1))
        code = (mb * (1 + 2 * (1 - sb))).astype(np.uint8)  # +1->1, -1->3
        # [K, O] -> [kcp, j, kk, h, col] -> fields
        cr = code.reshape(KCP, 2, 128, OH, OHW)
        c4 = cr.reshape(NQ, 2, 2, 128, OH, OHW)  # [q, kcp_sub, j, kk, h, col]
        byte = ((c4[:, 0, 0] << 6) | (c4[:, 0, 1] << 4)
                | (c4[:, 1, 0] << 2) | c4[:, 1, 1])   # [q, kk, h, col]
        wp_all[p] = byte.transpose(2, 1, 0, 3)        # [h, kk, q, col]
    return xs, wp_all, C


def _run(nc, in_maps, trace=False):
    from concourse import bass_utils
    return bass_utils.run_bass_kernel_spmd(
        nc, in_maps, core_ids=list(range(NCORES)), trace=trace
    )


def kernel(x, w, _trace=False, _return_results=False):
    x = np.asarray(x)
    w = np.asarray(w)
    assert x.shape == (B, I) and w.shape == (2, P, I, O)

    xs, wp_all, C = _unpack_inputs(x, w)

    if "nc" not in _CACHE:
        _CACHE["nc"] = _build_nc()
    nc = _CACHE["nc"]

    in_maps = []
    for c in range(NCORES):
        # groups for core c: [pl, h] -> wp_all[2c+pl, h]
        wp = np.ascontiguousarray(
            wp_all[2 * c:2 * c + PL].reshape(NGRP, 128, NQ, OHW)
        ).view(np.int8)
        in_maps.append({"xs": xs, "wp": wp})
    res = _run(nc, in_maps, trace=_trace)

    out = np.empty((P, B, O), np.int32)
    for c in range(NCORES):
        o = res.results[c]["out"]  # [NGRP, B, OHW] int8
        for pl in range(PL):
            full = np.concatenate(
                [o[pl * OH], o[pl * OH + 1]], axis=1)  # [B, O]
            out[c * PL + pl] = full.astype(np.int32) + C[c * PL + pl][None, :]
    if _return_results:
        return out, res
    return out
